# revision 2
# baseline (speedup 1.0000x reference)
"""GAT (2-layer, DGL-style) Bass kernel v2 for 8 Trainium2 NeuronCores.

Contract: kernel(**inputs) takes FULL unsharded inputs and returns the FULL
[N, NCLS] float32 output.

v2 design (vs the fp32 baseline):
- Nodes dst-sharded across cores; per-core edges sorted by local dst and
  packed into 128-slot windows (column-major 128-edge tiles).
- Stage A sharded: each core computes its own 12.5k-row feature table
  (row = [er(8)|el(8)|h(128)|pad] bf16, 512B), then one bf16 AllGather
  replicates the table.
- Per-window gathers are single multi-index indirect DMAs (int32 offsets,
  [128,k] offset AP): one by src from the replicated table (512B rows), one
  by dst from the core-local table for er (first 256B of each row).
- p = exp(leaky_relu(el+er)) elementwise (no pse/dx matmuls); msg = p*h in
  place; segment-sum via one-hot bf16 matmuls accumulating in PSUM, with p
  as extra rhs columns yielding the softmax denominator.
- Layer 2 repeats the same machinery on a [er2|el2|h2(64)|pad] 256B-row
  table (second AllGather).
"""

import math
from contextlib import ExitStack
from dataclasses import dataclass

import numpy as np
import ml_dtypes

from concourse import bacc, bass, mybir, tile
from concourse.bass import IndirectOffsetOnAxis
from concourse.bass_utils import run_bass_kernel_spmd
from concourse.masks import make_identity

f32 = mybir.dt.float32
bf16 = mybir.dt.bfloat16
i32 = mybir.dt.int32
ALU = mybir.AluOpType
ACTF = mybir.ActivationFunctionType

P = 128


@dataclass
class Cfg:
    N: int = 100000
    E: int = 1600000
    IN: int = 256
    HID: int = 128
    HEADS: int = 8
    DH: int = 16
    NCLS: int = 64
    C: int = 8
    SLOPE: float = 0.2

    @property
    def S(self):
        assert self.N % self.C == 0
        return self.N // self.C

    @property
    def ROW1(self):  # bf16 elems per L1 table row: er8|el8|h128|pad112
        return 256

    @property
    def ROW2(self):  # bf16 elems per L2 table row: er2|el2|h2(64)|pad62
        return 128

    @property
    def NW(self):
        return (self.S + P - 1) // P


def host_prep(cfg: Cfg, src: np.ndarray, dst: np.ndarray):
    """Sort edges by (dst shard, dst window); pack per-core index arrays
    [128, T] column-major with a window schedule shared by all cores."""
    S, NW, C = cfg.S, cfg.NW, cfg.C
    src = np.asarray(src).astype(np.int64)
    dst = np.asarray(dst).astype(np.int64)
    shard = dst // S
    per_core = []
    cnts = np.zeros((C, NW), dtype=np.int64)
    for c in range(C):
        m = shard == c
        s_c = src[m]
        dl = dst[m] - c * S
        order = np.argsort(dl, kind="stable")
        s_c, dl = s_c[order], dl[order]
        w = dl >> 7
        cnts[c] = np.bincount(w, minlength=NW)
        per_core.append((s_c, dl))
    kw = np.maximum(1, np.ceil(cnts.max(axis=0) / P)).astype(np.int64)
    offs = np.zeros(NW + 1, dtype=np.int64)
    offs[1:] = np.cumsum(kw)
    T = int(offs[-1])

    packs = []
    for c in range(C):
        s_c, dl = per_core[c]
        srcidx = np.zeros((P, T), np.int32)
        dstidx = np.zeros((P, T), np.int32)
        dstl = np.full((P, T), -1.0, np.float32)
        pos = 0
        for w in range(NW):
            n = int(cnts[c, w])
            k = int(kw[w])
            sb = np.zeros(k * P, np.int64)
            db = np.zeros(k * P, np.int64)
            lb = np.full(k * P, -1.0, np.float32)
            sb[:n] = s_c[pos : pos + n]
            db[:n] = dl[pos : pos + n]
            lb[:n] = (dl[pos : pos + n] - w * P).astype(np.float32)
            o = offs[w]
            srcidx[:, o : o + k] = sb.reshape(k, P).T
            dstidx[:, o : o + k] = db.reshape(k, P).T
            dstl[:, o : o + k] = lb.reshape(k, P).T
            pos += n
        dstlT = np.ascontiguousarray(dstl.T)
        packs.append((srcidx, dstidx, dstl.astype(ml_dtypes.bfloat16),
                      dstlT.astype(ml_dtypes.bfloat16)))
    return kw, offs, T, packs


def _ap(base: bass.AP, extra_offset_elems: int, dims):
    return bass.AP(
        tensor=base.tensor,
        offset=base.offset + extra_offset_elems,
        ap=[list(base.ap[0])] + [list(d) for d in dims],
    )


def build_program(cfg: Cfg, kw, offs, T, repeat: int = 1, debug_stage: int = 0):
    """debug_stage: 0=normal; 1=also dump t1loc; 2=also dump t1full+G/ER of
    window 0 of layer 1; 3=also dump t2loc."""
    nc = bacc.Bacc(
        "TRN2",
        target_bir_lowering=False,
        debug=False,
        enable_asserts=False,
        num_devices=cfg.C,
    )
    S, NW = cfg.S, cfg.NW
    IN, HID, HEADS, DH, NCLS = cfg.IN, cfg.HID, cfg.HEADS, cfg.DH, cfg.NCLS
    R1, R2 = cfg.ROW1, cfg.ROW2
    KC = IN // P  # 2 input-dim chunks
    F1 = HID + 2 * HEADS  # 144 useful cols of stage-A psum
    F2 = NCLS + 2  # 66 useful cols of layer-2 psum

    # ---- I/O ----
    xT_d = nc.dram_tensor("xT", [IN, cfg.N], f32, kind="ExternalInput").ap()
    W1_d = nc.dram_tensor("W1", [IN, HID], f32, kind="ExternalInput").ap()
    W1T_d = nc.dram_tensor("W1T", [HID, IN], f32, kind="ExternalInput").ap()
    alar1_d = nc.dram_tensor("alar1", [HID, 2 * HEADS], f32, kind="ExternalInput").ap()
    b1_d = nc.dram_tensor("b1", [HID], f32, kind="ExternalInput").ap()
    W2_d = nc.dram_tensor("W2", [HID, NCLS], f32, kind="ExternalInput").ap()
    W2T_d = nc.dram_tensor("W2T", [NCLS, HID], f32, kind="ExternalInput").ap()
    alar2_d = nc.dram_tensor("alar2", [NCLS, 2], f32, kind="ExternalInput").ap()
    b2_d = nc.dram_tensor("b2", [NCLS], f32, kind="ExternalInput").ap()
    srcidx_d = nc.dram_tensor("srcidx", [P, T], i32, kind="ExternalInput").ap()
    dstidx_d = nc.dram_tensor("dstidx", [P, T], i32, kind="ExternalInput").ap()
    dstl_d = nc.dram_tensor("dstl", [P, T], bf16, kind="ExternalInput").ap()
    dstlT_d = nc.dram_tensor("dstlT", [T, P], bf16, kind="ExternalInput").ap()
    out_d = nc.dram_tensor("out", [S, NCLS], f32, kind="ExternalOutput").ap()
    dbg_t1loc_d = dbg_t1full_d = dbg_G_d = dbg_ER_d = dbg_t2loc_d = None
    if debug_stage >= 1:
        dbg_t1loc_d = nc.dram_tensor("dbg_t1loc", [S, 256], bf16, kind="ExternalOutput").ap()
    if debug_stage >= 2:
        dbg_t1full_d = nc.dram_tensor("dbg_t1full", [cfg.N, 256], bf16, kind="ExternalOutput").ap()
        k0 = int(kw[0])
        dbg_G_d = nc.dram_tensor("dbg_G", [P, k0 * 256], bf16, kind="ExternalOutput").ap()
        dbg_ER_d = nc.dram_tensor("dbg_ER", [P, k0 * 128], bf16, kind="ExternalOutput").ap()
    if debug_stage >= 3:
        dbg_t2loc_d = nc.dram_tensor("dbg_t2loc", [S, 128], bf16, kind="ExternalOutput").ap()

    # ---- internal DRAM ----
    t1loc_d = nc.dram_tensor("t1loc", [S, R1], bf16).ap()
    t1full_d = nc.dram_tensor("t1full", [cfg.N, R1], bf16, addr_space="Shared").ap()
    t2loc_d = nc.dram_tensor("t2loc", [S, R2], bf16).ap()
    t2full_d = nc.dram_tensor("t2full", [cfg.N, R2], bf16, addr_space="Shared").ap()

    with tile.TileContext(nc) as tc, ExitStack() as octx:
        const = octx.enter_context(tc.tile_pool(name="const", bufs=1))

        # ---- constants ----
        identity = const.tile([P, P], f32)
        make_identity(nc, identity[:])
        identity_b = const.tile([P, P], bf16)
        nc.vector.tensor_copy(identity_b[:], identity[:])
        iota_i = const.tile([P, P], i32)
        nc.gpsimd.iota(iota_i[:], pattern=[[1, P]], base=0, channel_multiplier=0)
        iota_b = const.tile([P, P], bf16)
        nc.vector.tensor_copy(iota_b[:], iota_i[:])
        ones_row = const.tile([1, P], f32)
        nc.vector.memset(ones_row[:], 1.0)
        ones_row_b = const.tile([1, P], bf16)
        nc.vector.memset(ones_row_b[:], 1.0)
        iota_ci = const.tile([P, 1], i32)
        nc.gpsimd.iota(iota_ci[:], pattern=[[0, 1]], base=0, channel_multiplier=1)
        iota_c = const.tile([P, 1], f32)
        nc.vector.tensor_copy(iota_c[:], iota_ci[:])

        W1e = []
        for kc in range(KC):
            w1e_t = const.tile([P, F1], bf16, tag=f"w1e{kc}", name=f"w1e{kc}")
            W1e.append(w1e_t)
        W2e = const.tile([P, F2], bf16)
        bias1 = const.tile([P, HID], f32)
        bias2 = const.tile([P, NCLS], f32)

        with ExitStack() as ictx:
            ipool = ictx.enter_context(tc.tile_pool(name="init_sb", bufs=2))
            ipsum = ictx.enter_context(tc.tile_pool(name="init_ps", bufs=2, space="PSUM"))

            # bias rows broadcast down partitions via ones-matmul
            b1row = ipool.tile([1, HID], f32)
            nc.sync.dma_start(out=b1row[:], in_=b1_d[:].rearrange("(a b) -> a b", a=1))
            b2row = ipool.tile([1, NCLS], f32)
            nc.sync.dma_start(out=b2row[:], in_=b2_d[:].rearrange("(a b) -> a b", a=1))
            pb1 = ipsum.tile([P, HID], f32)
            nc.tensor.matmul(pb1[:], lhsT=ones_row[:], rhs=b1row[:], start=True, stop=True)
            nc.vector.tensor_copy(bias1[:], pb1[:])
            pb2 = ipsum.tile([P, NCLS], f32)
            nc.tensor.matmul(pb2[:], lhsT=ones_row[:], rhs=b2row[:], start=True, stop=True)
            nc.vector.tensor_copy(bias2[:], pb2[:])

            # W2ext = [W2 | W2@al2 | W2@ar2]  [HID, F2] -> bf16
            w2sb = ipool.tile([P, NCLS], f32, tag="w2sb")
            nc.sync.dma_start(out=w2sb[:HID, :], in_=W2_d[:, :])
            w2t_sb = ipool.tile([NCLS, HID], f32, tag="w2t")
            nc.sync.dma_start(out=w2t_sb[:], in_=W2T_d[:, :])
            alar2_sb = ipool.tile([NCLS, 2], f32, tag="alar2")
            nc.sync.dma_start(out=alar2_sb[:], in_=alar2_d[:, :])
            pw2 = ipsum.tile([P, 2], f32, tag="pw2")
            nc.tensor.matmul(pw2[:HID, :], lhsT=w2t_sb[:NCLS, :], rhs=alar2_sb[:NCLS, :], start=True, stop=True)
            nc.vector.tensor_copy(W2e[:HID, 0:NCLS], w2sb[:HID, :])
            nc.vector.tensor_copy(W2e[:HID, NCLS:F2], pw2[:HID, :])

            # W1ext = [W1 | W1@alar1]  [IN, F1] -> bf16 chunk tiles
            alar1_sb = ipool.tile([HID, 2 * HEADS], f32, tag="alar1")
            nc.sync.dma_start(out=alar1_sb[:], in_=alar1_d[:, :])
            for kc in range(KC):
                w1sb = ipool.tile([P, HID], f32, tag="w1sb")
                nc.sync.dma_start(out=w1sb[:, :], in_=W1_d[kc * P : (kc + 1) * P, :])
                w1t_sb = ipool.tile([HID, P], f32, tag="w1t")
                nc.sync.dma_start(out=w1t_sb[:, :], in_=W1T_d[:, kc * P : (kc + 1) * P])
                pwe = ipsum.tile([P, 2 * HEADS], f32, tag="pwe")
                nc.tensor.matmul(pwe[:, :], lhsT=w1t_sb[:HID, :], rhs=alar1_sb[:HID, :], start=True, stop=True)
                nc.vector.tensor_copy(W1e[kc][:, 0:HID], w1sb[:, :])
                nc.vector.tensor_copy(W1e[kc][:, HID:F1], pwe[:, :])

        sbase = nc.partition_id() * S

        # ---- stage A: own-shard table1 rows [er|el|h|pad] ----
        def stage_a(actx: ExitStack):
            ax = actx.enter_context(tc.tile_pool(name="a_x", bufs=3))
            ast = actx.enter_context(tc.tile_pool(name="a_st", bufs=3))
            aps = actx.enter_context(tc.tile_pool(name="a_ps", bufs=3, space="PSUM"))
            for t in range(NW):
                base = t * P
                ns = min(P, S - base)
                xa = ax.tile([P, KC * P], f32, tag="xa")
                for kc in range(KC):
                    nc.sync.dma_start(
                        out=xa[:, kc * P : kc * P + ns],
                        in_=xT_d[kc * P : (kc + 1) * P, bass.ds(sbase + base, ns)],
                    )
                xb = ax.tile([P, KC * P], bf16, tag="xb")
                nc.vector.tensor_copy(xb[:], xa[:])
                ps = aps.tile([P, F1], f32, tag="aps")
                for kc in range(KC):
                    nc.tensor.matmul(
                        ps[:ns, :],
                        lhsT=xb[:, kc * P : kc * P + ns],
                        rhs=W1e[kc][:, :],
                        start=(kc == 0),
                        stop=(kc == KC - 1),
                    )
                tl = ast.tile([P, R1], bf16, tag="tl")
                nc.vector.tensor_copy(tl[:ns, 0:HEADS], ps[:ns, HID + HEADS : F1])
                nc.vector.tensor_copy(tl[:ns, HEADS : 2 * HEADS], ps[:ns, HID : HID + HEADS])
                nc.vector.tensor_copy(tl[:ns, 2 * HEADS : 2 * HEADS + HID], ps[:ns, 0:HID])
                nc.vector.memset(tl[:ns, 2 * HEADS + HID : R1], 0.0)
                nc.sync.dma_start(out=t1loc_d[base : base + ns, :], in_=tl[:ns, :])

        # ---- edge phase ----
        def edge_phase(layer: int, ectx: ExitStack):
            if layer == 1:
                TBL, LOC, RW, NH, MW = t1full_d, t1loc_d, R1, HEADS, HID + HEADS
            else:
                TBL, LOC, RW, NH, MW = t2full_d, t2loc_d, R2, 1, NCLS + 1
            # row layout: [er(NH) | el(NH) | h(MW-NH) | pad]
            ip = ectx.enter_context(tc.tile_pool(name=f"e{layer}_idx", bufs=8))
            gp = ectx.enter_context(tc.tile_pool(name=f"e{layer}_g", bufs=4))
            ep = ectx.enter_context(tc.tile_pool(name=f"e{layer}_er", bufs=3))
            op = ectx.enter_context(tc.tile_pool(name=f"e{layer}_oh", bufs=3))
            sp = ectx.enter_context(tc.tile_pool(name=f"e{layer}_s", bufs=4))
            wp = ectx.enter_context(tc.tile_pool(name=f"e{layer}_w", bufs=4))
            pp = ectx.enter_context(tc.tile_pool(name=f"e{layer}_ps", bufs=2, space="PSUM"))
            pep = ectx.enter_context(tc.tile_pool(name=f"e{layer}_dx", bufs=1, space="PSUM"))
            pep2 = ectx.enter_context(tc.tile_pool(name=f"e{layer}_pse", bufs=1, space="PSUM"))
            if layer == 1:
                ptp = ectx.enter_context(tc.tile_pool(name="e1_pst", bufs=2, space="PSUM"))
                php = ectx.enter_context(tc.tile_pool(name="e1_ph", bufs=2, space="PSUM"))
            ERW = 128  # elems gathered per er-row read (256B)

            for w in range(NW):
                k = int(kw[w])
                o = int(offs[w])
                base = w * P
                ns = min(P, S - base)

                si = ip.tile([P, k], i32, tag="si")
                nc.sync.dma_start(out=si[:], in_=srcidx_d[:, o : o + k])
                di = ip.tile([P, k], i32, tag="di")
                nc.sync.dma_start(out=di[:], in_=dstidx_d[:, o : o + k])
                dl = ip.tile([P, k], bf16, tag="dl")
                nc.sync.dma_start(out=dl[:], in_=dstl_d[:, o : o + k])

                dlT = ip.tile([1, k * P], bf16, tag="dlT")
                nc.sync.dma_start(
                    out=dlT[:],
                    in_=bass.AP(tensor=dstlT_d.tensor, offset=o * P,
                                ap=[[1, 1], [1, k * P]]),
                )

                # per-column 128-row gathers (one offset per partition)
                G = gp.tile([P, k * RW], bf16, tag="G")
                for j in range(k):
                    nc.gpsimd.indirect_dma_start(
                        out=G[:, j * RW : (j + 1) * RW],
                        out_offset=None,
                        in_=TBL[:, :],
                        in_offset=IndirectOffsetOnAxis(ap=si[:, j : j + 1], axis=0),
                    )
                if debug_stage >= 2 and layer == 1 and w == 0:
                    nc.sync.dma_start(out=dbg_G_d[:, :], in_=G[:])

                # er for this window's dst slots: sequential rows of LOC
                erwin = ep.tile([P, NH], bf16, tag="erwin")
                nc.vector.memset(erwin[:], 0.0)
                nc.sync.dma_start(out=erwin[:ns, :], in_=LOC[base : base + ns, 0:NH])

                # OHT[slot, edge] = (dstl[edge] == slot): broadcast dstlT down
                # partitions via ones-matmul, compare to per-partition iota
                OHT = op.tile([P, k * P], bf16, tag="OHT")
                for c0 in range(0, k * P, 512):
                    cw = min(512, k * P - c0)
                    dx = pep.tile([P, 512], f32, tag="dx")
                    nc.tensor.matmul(
                        dx[:, :cw], lhsT=ones_row_b[:], rhs=dlT[:, c0 : c0 + cw],
                        start=True, stop=True,
                    )
                    nc.vector.tensor_scalar(
                        out=OHT[:, c0 : c0 + cw], in0=dx[:, :cw],
                        scalar1=iota_c[:, 0:1], scalar2=None, op0=ALU.is_equal,
                    )

                # er per edge via PE: pse = OHT_j^T @ erwin
                pse = pep2.tile([P, k * NH], f32, tag="pse")
                for j in range(k):
                    nc.tensor.matmul(
                        pse[:, j * NH : (j + 1) * NH],
                        lhsT=OHT[:, j * P : (j + 1) * P],
                        rhs=erwin[:],
                        start=True, stop=True,
                    )

                # e = el[src] + er[dst]
                el_view = _ap(G[:], NH, [[RW, k], [1, NH]])
                et = sp.tile([P, k * NH], f32, tag="et")
                nc.vector.tensor_tensor(out=et[:], in0=el_view, in1=pse[:], op=ALU.add)
                ea = sp.tile([P, k * NH], f32, tag="ea")
                nc.scalar.activation(ea[:], et[:], ACTF.Exp)
                eb = sp.tile([P, k * NH], f32, tag="eb")
                nc.scalar.activation(eb[:], et[:], ACTF.Exp, scale=cfg.SLOPE)
                # p -> G's el cols (bf16)
                nc.vector.tensor_tensor(out=el_view, in0=ea[:], in1=eb[:], op=ALU.max)

                # msg = p * h in place
                if layer == 1:
                    h_view = _ap(G[:], 2 * NH, [[RW, k], [DH, HEADS], [1, DH]])
                    p_view = _ap(G[:], NH, [[RW, k], [1, HEADS], [0, DH]])
                else:
                    h_view = _ap(G[:], 2, [[RW, k], [1, NCLS]])
                    p_view = _ap(G[:], 1, [[RW, k], [0, NCLS]])
                nc.vector.tensor_tensor(out=h_view, in0=h_view, in1=p_view, op=ALU.mult)

                # one-hot [edges, slots] per column
                OH = op.tile([P, k * P], bf16, tag="OH")
                nc.vector.tensor_tensor(
                    out=_ap(OH[:], 0, [[P, k], [1, P]]),
                    in0=_ap(iota_b[:], 0, [[0, k], [1, P]]),
                    in1=_ap(dl[:], 0, [[1, k], [0, P]]),
                    op=ALU.is_equal,
                )

                # scatter: psum[slot, 0:NH]=sum p, [NH:MW]=sum p*h
                ps = pp.tile([P, MW], f32, tag="eps")
                for j in range(k):
                    nc.tensor.matmul(
                        ps[:, :],
                        lhsT=OH[:, j * P : (j + 1) * P],
                        rhs=G[:, j * RW + NH : j * RW + NH + MW],
                        start=(j == 0),
                        stop=(j == k - 1),
                    )

                scl = sp.tile([P, NH], f32, tag="scl")
                nc.vector.tensor_scalar(
                    out=scl[:], in0=ps[:, 0:NH], scalar1=1e-30, scalar2=None, op0=ALU.max
                )
                rs = sp.tile([P, NH], f32, tag="rs")
                nc.vector.reciprocal(rs[:], scl[:])

                if layer == 1:
                    h1 = wp.tile([P, HID], f32, tag="h1")
                    nc.vector.tensor_tensor(
                        out=_ap(h1[:], 0, [[DH, HEADS], [1, DH]]),
                        in0=_ap(ps[:], NH, [[DH, HEADS], [1, DH]]),
                        in1=_ap(rs[:], 0, [[1, HEADS], [0, DH]]),
                        op=ALU.mult,
                    )
                    nc.vector.tensor_tensor(out=h1[:], in0=h1[:], in1=bias1[:], op=ALU.add)
                    cur = h1
                    for r in range(2):
                        tmin = wp.tile([P, HID], f32, tag=f"tmin{r}")
                        nc.vector.tensor_scalar(
                            out=tmin[:], in0=cur[:], scalar1=0.0, scalar2=None, op0=ALU.min
                        )
                        nc.scalar.activation(tmin[:], tmin[:], ACTF.Exp)
                        nc.vector.tensor_scalar(
                            out=tmin[:], in0=tmin[:], scalar1=-1.0, scalar2=None, op0=ALU.add
                        )
                        nxt = wp.tile([P, HID], f32, tag=f"helu{r}")
                        nc.vector.tensor_tensor(out=nxt[:], in0=cur[:], in1=tmin[:], op=ALU.max)
                        cur = nxt
                    # h2 = cur.T-transpose (fp32) then @ W2e (bf16)
                    pt = ptp.tile([P, P], f32, tag="pt")
                    nc.tensor.transpose(pt[:], cur[:], identity[:])
                    hT = wp.tile([P, P], bf16, tag="hT")
                    nc.vector.tensor_copy(hT[:], pt[:])
                    ps2 = php.tile([P, F2], f32, tag="ph2")
                    nc.tensor.matmul(
                        ps2[:, :], lhsT=hT[:HID, :], rhs=W2e[:HID, :], start=True, stop=True
                    )
                    t2 = wp.tile([P, R2], bf16, tag="t2")
                    nc.vector.tensor_copy(t2[:ns, 0:1], ps2[:ns, NCLS + 1 : NCLS + 2])
                    nc.vector.tensor_copy(t2[:ns, 1:2], ps2[:ns, NCLS : NCLS + 1])
                    nc.vector.tensor_copy(t2[:ns, 2 : 2 + NCLS], ps2[:ns, 0:NCLS])
                    nc.vector.memset(t2[:ns, 2 + NCLS : R2], 0.0)
                    nc.sync.dma_start(out=t2loc_d[base : base + ns, :], in_=t2[:ns, :])
                else:
                    o2 = wp.tile([P, NCLS], f32, tag="o2")
                    nc.vector.tensor_scalar(
                        out=o2[:], in0=ps[:, 1 : 1 + NCLS], scalar1=rs[:, 0:1], scalar2=None, op0=ALU.mult
                    )
                    nc.vector.tensor_tensor(out=o2[:], in0=o2[:], in1=bias2[:], op=ALU.add)
                    nc.sync.dma_start(out=out_d[base : base + ns, :], in_=o2[:ns, :])

        for _rep in range(repeat):
            with ExitStack() as actx:
                stage_a(actx)

            nc.gpsimd.collective_compute(
                "AllGather", ALU.bypass, replica_groups=[list(range(cfg.C))],
                ins=[t1loc_d[:, :]], outs=[t1full_d[:, :]],
            )
            if debug_stage >= 1:
                nc.sync.dma_start(out=dbg_t1loc_d[:, :], in_=t1loc_d[:, :])
            if debug_stage >= 2:
                nc.sync.dma_start(out=dbg_t1full_d[:, :], in_=t1full_d[:, :])

            with ExitStack() as e1ctx:
                edge_phase(1, e1ctx)
            if debug_stage >= 3:
                nc.sync.dma_start(out=dbg_t2loc_d[:, :], in_=t2loc_d[:, :])

            nc.gpsimd.collective_compute(
                "AllGather", ALU.bypass, replica_groups=[list(range(cfg.C))],
                ins=[t2loc_d[:, :]], outs=[t2full_d[:, :]],
            )

            with ExitStack() as e2ctx:
                edge_phase(2, e2ctx)

    nc.compile()
    return nc


def make_inmaps(cfg: Cfg, inputs: dict, packs):
    x = np.asarray(inputs["x"], np.float32)
    W1 = np.asarray(inputs["W1"], np.float32)
    al1 = np.asarray(inputs["al1"], np.float32)
    ar1 = np.asarray(inputs["ar1"], np.float32)
    b1 = np.asarray(inputs["b1"], np.float32)
    W2 = np.asarray(inputs["W2"], np.float32)
    al2 = np.asarray(inputs["al2"], np.float32)
    ar2 = np.asarray(inputs["ar2"], np.float32)
    b2 = np.asarray(inputs["b2"], np.float32)

    xT = np.ascontiguousarray(x.T)
    W1T = np.ascontiguousarray(W1.T)
    W2T = np.ascontiguousarray(W2.T)
    alar1 = np.zeros((cfg.HID, 2 * cfg.HEADS), np.float32)
    for h in range(cfg.HEADS):
        alar1[h * cfg.DH : (h + 1) * cfg.DH, h] = al1[h]
        alar1[h * cfg.DH : (h + 1) * cfg.DH, cfg.HEADS + h] = ar1[h]
    alar2 = np.stack([al2[0], ar2[0]], axis=1).astype(np.float32)

    in_maps = []
    for c in range(cfg.C):
        srcidx, dstidx, dstl, dstlT = packs[c]
        in_maps.append(
            {
                "xT": xT, "W1": W1, "W1T": W1T, "alar1": alar1, "b1": b1,
                "W2": W2, "W2T": W2T, "alar2": alar2, "b2": b2,
                "srcidx": srcidx, "dstidx": dstidx, "dstl": dstl,
                "dstlT": dstlT,
            }
        )
    return in_maps


def run(cfg: Cfg, inputs: dict, trace: bool = False):
    kw, offs, T, packs = host_prep(cfg, inputs["src"], inputs["dst"])
    nc = build_program(cfg, kw, offs, T)
    in_maps = make_inmaps(cfg, inputs, packs)
    res = run_bass_kernel_spmd(nc, in_maps, core_ids=list(range(cfg.C)), trace=trace)
    out = np.concatenate([res.results[c]["out"] for c in range(cfg.C)], axis=0)
    return out, res


def kernel(**inputs) -> np.ndarray:
    cfg = Cfg()
    out, _ = run(cfg, inputs)
    return out.astype(np.float32)


# revision 3
# speedup vs baseline: 1.0382x; 1.0382x over previous
"""GAT (2-layer, DGL-style) Bass kernel v2 for 8 Trainium2 NeuronCores.

Contract: kernel(**inputs) takes FULL unsharded inputs and returns the FULL
[N, NCLS] float32 output.

v2 design (vs the fp32 baseline):
- Nodes dst-sharded across cores; per-core edges sorted by local dst and
  packed into 128-slot windows (column-major 128-edge tiles).
- Stage A sharded: each core computes its own 12.5k-row feature table
  (row = [er(8)|el(8)|h(128)|pad] bf16, 512B), then one bf16 AllGather
  replicates the table.
- Per-window gathers are single multi-index indirect DMAs (int32 offsets,
  [128,k] offset AP): one by src from the replicated table (512B rows), one
  by dst from the core-local table for er (first 256B of each row).
- p = exp(leaky_relu(el+er)) elementwise (no pse/dx matmuls); msg = p*h in
  place; segment-sum via one-hot bf16 matmuls accumulating in PSUM, with p
  as extra rhs columns yielding the softmax denominator.
- Layer 2 repeats the same machinery on a [er2|el2|h2(64)|pad] 256B-row
  table (second AllGather).
"""

import math
from contextlib import ExitStack
from dataclasses import dataclass

import numpy as np
import ml_dtypes

from concourse import bacc, bass, mybir, tile
from concourse.bass import IndirectOffsetOnAxis
from concourse.bass_utils import run_bass_kernel_spmd
from concourse.masks import make_identity

f32 = mybir.dt.float32
bf16 = mybir.dt.bfloat16
i32 = mybir.dt.int32
ALU = mybir.AluOpType
ACTF = mybir.ActivationFunctionType

P = 128


@dataclass
class Cfg:
    N: int = 100000
    E: int = 1600000
    IN: int = 256
    HID: int = 128
    HEADS: int = 8
    DH: int = 16
    NCLS: int = 64
    C: int = 8
    SLOPE: float = 0.2

    @property
    def S(self):
        assert self.N % self.C == 0
        return self.N // self.C

    @property
    def ROW1(self):  # bf16 elems per L1 table row: er8|el8|h128|pad112
        return 256

    @property
    def ROW2(self):  # bf16 elems per L2 table row: er2|el2|h2(64)|pad62
        return 128

    @property
    def NW(self):
        return (self.S + P - 1) // P


def host_prep(cfg: Cfg, src: np.ndarray, dst: np.ndarray):
    """Sort edges by (dst shard, dst window); pack per-core index arrays
    [128, T] column-major with a window schedule shared by all cores."""
    S, NW, C = cfg.S, cfg.NW, cfg.C
    src = np.asarray(src).astype(np.int64)
    dst = np.asarray(dst).astype(np.int64)
    shard = dst // S
    per_core = []
    cnts = np.zeros((C, NW), dtype=np.int64)
    for c in range(C):
        m = shard == c
        s_c = src[m]
        dl = dst[m] - c * S
        order = np.argsort(dl, kind="stable")
        s_c, dl = s_c[order], dl[order]
        w = dl >> 7
        cnts[c] = np.bincount(w, minlength=NW)
        per_core.append((s_c, dl))
    kw = np.maximum(1, np.ceil(cnts.max(axis=0) / P)).astype(np.int64)
    offs = np.zeros(NW + 1, dtype=np.int64)
    offs[1:] = np.cumsum(kw)
    T = int(offs[-1])

    packs = []
    for c in range(C):
        s_c, dl = per_core[c]
        srcidx = np.zeros((P, T), np.int32)
        dstidx = np.zeros((P, T), np.int32)
        dstl = np.full((P, T), -1.0, np.float32)
        pos = 0
        for w in range(NW):
            n = int(cnts[c, w])
            k = int(kw[w])
            sb = np.zeros(k * P, np.int64)
            db = np.zeros(k * P, np.int64)
            lb = np.full(k * P, -1.0, np.float32)
            sb[:n] = s_c[pos : pos + n]
            db[:n] = dl[pos : pos + n]
            lb[:n] = (dl[pos : pos + n] - w * P).astype(np.float32)
            o = offs[w]
            srcidx[:, o : o + k] = sb.reshape(k, P).T
            dstidx[:, o : o + k] = db.reshape(k, P).T
            dstl[:, o : o + k] = lb.reshape(k, P).T
            pos += n
        dstlT = np.ascontiguousarray(dstl.T)
        packs.append((srcidx, dstidx, dstl.astype(ml_dtypes.bfloat16),
                      dstlT.astype(ml_dtypes.bfloat16)))
    return kw, offs, T, packs


def _ap(base: bass.AP, extra_offset_elems: int, dims):
    return bass.AP(
        tensor=base.tensor,
        offset=base.offset + extra_offset_elems,
        ap=[list(base.ap[0])] + [list(d) for d in dims],
    )


def build_program(cfg: Cfg, kw, offs, T, repeat: int = 1, debug_stage: int = 0):
    """debug_stage: 0=normal; 1=also dump t1loc; 2=also dump t1full+G/ER of
    window 0 of layer 1; 3=also dump t2loc."""
    nc = bacc.Bacc(
        "TRN2",
        target_bir_lowering=False,
        debug=False,
        enable_asserts=False,
        num_devices=cfg.C,
    )
    S, NW = cfg.S, cfg.NW
    IN, HID, HEADS, DH, NCLS = cfg.IN, cfg.HID, cfg.HEADS, cfg.DH, cfg.NCLS
    R1, R2 = cfg.ROW1, cfg.ROW2
    KC = IN // P  # 2 input-dim chunks
    F1 = HID + 2 * HEADS  # 144 useful cols of stage-A psum
    F2 = NCLS + 2  # 66 useful cols of layer-2 psum

    # ---- I/O ----
    xT_d = nc.dram_tensor("xT", [IN, cfg.N], f32, kind="ExternalInput").ap()
    W1_d = nc.dram_tensor("W1", [IN, HID], f32, kind="ExternalInput").ap()
    W1T_d = nc.dram_tensor("W1T", [HID, IN], f32, kind="ExternalInput").ap()
    alar1_d = nc.dram_tensor("alar1", [HID, 2 * HEADS], f32, kind="ExternalInput").ap()
    b1_d = nc.dram_tensor("b1", [HID], f32, kind="ExternalInput").ap()
    W2_d = nc.dram_tensor("W2", [HID, NCLS], f32, kind="ExternalInput").ap()
    W2T_d = nc.dram_tensor("W2T", [NCLS, HID], f32, kind="ExternalInput").ap()
    alar2_d = nc.dram_tensor("alar2", [NCLS, 2], f32, kind="ExternalInput").ap()
    b2_d = nc.dram_tensor("b2", [NCLS], f32, kind="ExternalInput").ap()
    srcidx_d = nc.dram_tensor("srcidx", [P, T], i32, kind="ExternalInput").ap()
    dstidx_d = nc.dram_tensor("dstidx", [P, T], i32, kind="ExternalInput").ap()
    dstl_d = nc.dram_tensor("dstl", [P, T], bf16, kind="ExternalInput").ap()
    dstlT_d = nc.dram_tensor("dstlT", [T, P], bf16, kind="ExternalInput").ap()
    out_d = nc.dram_tensor("out", [S, NCLS], f32, kind="ExternalOutput").ap()
    dbg_t1loc_d = dbg_t1full_d = dbg_G_d = dbg_ER_d = dbg_t2loc_d = None
    if debug_stage >= 1:
        dbg_t1loc_d = nc.dram_tensor("dbg_t1loc", [S, 256], bf16, kind="ExternalOutput").ap()
    if debug_stage >= 2:
        dbg_t1full_d = nc.dram_tensor("dbg_t1full", [cfg.N, 256], bf16, kind="ExternalOutput").ap()
        k0 = int(kw[0])
        dbg_G_d = nc.dram_tensor("dbg_G", [P, k0 * 256], bf16, kind="ExternalOutput").ap()
        dbg_ER_d = nc.dram_tensor("dbg_ER", [P, k0 * 128], bf16, kind="ExternalOutput").ap()
    if debug_stage >= 3:
        dbg_t2loc_d = nc.dram_tensor("dbg_t2loc", [S, 128], bf16, kind="ExternalOutput").ap()

    # ---- internal DRAM ----
    t1loc_d = nc.dram_tensor("t1loc", [S, R1], bf16).ap()
    t1full_d = nc.dram_tensor("t1full", [cfg.N, R1], bf16, addr_space="Shared").ap()
    t2loc_d = nc.dram_tensor("t2loc", [S, R2], bf16).ap()
    t2full_d = nc.dram_tensor("t2full", [cfg.N, R2], bf16, addr_space="Shared").ap()

    with tile.TileContext(nc) as tc, ExitStack() as octx:
        const = octx.enter_context(tc.tile_pool(name="const", bufs=1))

        # ---- constants ----
        identity = const.tile([P, P], f32)
        make_identity(nc, identity[:])
        identity_b = const.tile([P, P], bf16)
        nc.vector.tensor_copy(identity_b[:], identity[:])
        iota_i = const.tile([P, P], i32)
        nc.gpsimd.iota(iota_i[:], pattern=[[1, P]], base=0, channel_multiplier=0)
        iota_b = const.tile([P, P], bf16)
        nc.vector.tensor_copy(iota_b[:], iota_i[:])
        ones_row = const.tile([1, P], f32)
        nc.vector.memset(ones_row[:], 1.0)
        ones_row_b = const.tile([1, P], bf16)
        nc.vector.memset(ones_row_b[:], 1.0)
        iota_ci = const.tile([P, 1], i32)
        nc.gpsimd.iota(iota_ci[:], pattern=[[0, 1]], base=0, channel_multiplier=1)
        iota_c = const.tile([P, 1], f32)
        nc.vector.tensor_copy(iota_c[:], iota_ci[:])

        W1e = []
        for kc in range(KC):
            w1e_t = const.tile([P, F1], bf16, tag=f"w1e{kc}", name=f"w1e{kc}")
            W1e.append(w1e_t)
        W2e = const.tile([P, F2], bf16)
        bias1 = const.tile([P, HID], f32)
        bias2 = const.tile([P, NCLS], f32)

        with ExitStack() as ictx:
            ipool = ictx.enter_context(tc.tile_pool(name="init_sb", bufs=2))
            ipsum = ictx.enter_context(tc.tile_pool(name="init_ps", bufs=2, space="PSUM"))

            # bias rows broadcast down partitions via ones-matmul
            b1row = ipool.tile([1, HID], f32)
            nc.sync.dma_start(out=b1row[:], in_=b1_d[:].rearrange("(a b) -> a b", a=1))
            b2row = ipool.tile([1, NCLS], f32)
            nc.sync.dma_start(out=b2row[:], in_=b2_d[:].rearrange("(a b) -> a b", a=1))
            pb1 = ipsum.tile([P, HID], f32)
            nc.tensor.matmul(pb1[:], lhsT=ones_row[:], rhs=b1row[:], start=True, stop=True)
            nc.vector.tensor_copy(bias1[:], pb1[:])
            pb2 = ipsum.tile([P, NCLS], f32)
            nc.tensor.matmul(pb2[:], lhsT=ones_row[:], rhs=b2row[:], start=True, stop=True)
            nc.vector.tensor_copy(bias2[:], pb2[:])

            # W2ext = [W2 | W2@al2 | W2@ar2]  [HID, F2] -> bf16
            w2sb = ipool.tile([P, NCLS], f32, tag="w2sb")
            nc.sync.dma_start(out=w2sb[:HID, :], in_=W2_d[:, :])
            w2t_sb = ipool.tile([NCLS, HID], f32, tag="w2t")
            nc.sync.dma_start(out=w2t_sb[:], in_=W2T_d[:, :])
            alar2_sb = ipool.tile([NCLS, 2], f32, tag="alar2")
            nc.sync.dma_start(out=alar2_sb[:], in_=alar2_d[:, :])
            pw2 = ipsum.tile([P, 2], f32, tag="pw2")
            nc.tensor.matmul(pw2[:HID, :], lhsT=w2t_sb[:NCLS, :], rhs=alar2_sb[:NCLS, :], start=True, stop=True)
            nc.vector.tensor_copy(W2e[:HID, 0:NCLS], w2sb[:HID, :])
            nc.vector.tensor_copy(W2e[:HID, NCLS:F2], pw2[:HID, :])

            # W1ext = [W1 | W1@alar1]  [IN, F1] -> bf16 chunk tiles
            alar1_sb = ipool.tile([HID, 2 * HEADS], f32, tag="alar1")
            nc.sync.dma_start(out=alar1_sb[:], in_=alar1_d[:, :])
            for kc in range(KC):
                w1sb = ipool.tile([P, HID], f32, tag="w1sb")
                nc.sync.dma_start(out=w1sb[:, :], in_=W1_d[kc * P : (kc + 1) * P, :])
                w1t_sb = ipool.tile([HID, P], f32, tag="w1t")
                nc.sync.dma_start(out=w1t_sb[:, :], in_=W1T_d[:, kc * P : (kc + 1) * P])
                pwe = ipsum.tile([P, 2 * HEADS], f32, tag="pwe")
                nc.tensor.matmul(pwe[:, :], lhsT=w1t_sb[:HID, :], rhs=alar1_sb[:HID, :], start=True, stop=True)
                nc.vector.tensor_copy(W1e[kc][:, 0:HID], w1sb[:, :])
                nc.vector.tensor_copy(W1e[kc][:, HID:F1], pwe[:, :])

        sbase = nc.partition_id() * S

        # ---- stage A: own-shard table1 rows [er|el|h|pad] ----
        def stage_a(actx: ExitStack):
            ax = actx.enter_context(tc.tile_pool(name="a_x", bufs=3))
            ast = actx.enter_context(tc.tile_pool(name="a_st", bufs=3))
            aps = actx.enter_context(tc.tile_pool(name="a_ps", bufs=3, space="PSUM"))
            for t in range(NW):
                base = t * P
                ns = min(P, S - base)
                xa = ax.tile([P, KC * P], f32, tag="xa")
                for kc in range(KC):
                    nc.sync.dma_start(
                        out=xa[:, kc * P : kc * P + ns],
                        in_=xT_d[kc * P : (kc + 1) * P, bass.ds(sbase + base, ns)],
                    )
                xb = ax.tile([P, KC * P], bf16, tag="xb")
                nc.vector.tensor_copy(xb[:], xa[:])
                ps = aps.tile([P, F1], f32, tag="aps")
                for kc in range(KC):
                    nc.tensor.matmul(
                        ps[:ns, :],
                        lhsT=xb[:, kc * P : kc * P + ns],
                        rhs=W1e[kc][:, :],
                        start=(kc == 0),
                        stop=(kc == KC - 1),
                    )
                tl = ast.tile([P, R1], bf16, tag="tl")
                nc.vector.tensor_copy(tl[:ns, 0:HEADS], ps[:ns, HID + HEADS : F1])
                nc.vector.tensor_copy(tl[:ns, HEADS : 2 * HEADS], ps[:ns, HID : HID + HEADS])
                nc.vector.tensor_copy(tl[:ns, 2 * HEADS : 2 * HEADS + HID], ps[:ns, 0:HID])
                nc.vector.memset(tl[:ns, 2 * HEADS + HID : R1], 0.0)
                nc.sync.dma_start(out=t1loc_d[base : base + ns, :], in_=tl[:ns, :])

        # ---- edge phase ----
        def edge_phase(layer: int, ectx: ExitStack):
            if layer == 1:
                TBL, LOC, RW, NH, MW = t1full_d, t1loc_d, R1, HEADS, HID + HEADS
            else:
                TBL, LOC, RW, NH, MW = t2full_d, t2loc_d, R2, 1, NCLS + 1
            # row layout: [er(NH) | el(NH) | h(MW-NH) | pad]
            ip = ectx.enter_context(tc.tile_pool(name=f"e{layer}_idx", bufs=8))
            gp = ectx.enter_context(tc.tile_pool(name=f"e{layer}_g", bufs=5))
            ep = ectx.enter_context(tc.tile_pool(name=f"e{layer}_er", bufs=3))
            op = ectx.enter_context(tc.tile_pool(name=f"e{layer}_oh", bufs=4))
            sp = ectx.enter_context(tc.tile_pool(name=f"e{layer}_s", bufs=4))
            wp = ectx.enter_context(tc.tile_pool(name=f"e{layer}_w", bufs=4))
            pp = ectx.enter_context(tc.tile_pool(name=f"e{layer}_ps", bufs=3, space="PSUM"))
            pep = ectx.enter_context(tc.tile_pool(name=f"e{layer}_dx", bufs=1, space="PSUM"))
            pep2 = ectx.enter_context(tc.tile_pool(name=f"e{layer}_pse", bufs=1, space="PSUM"))
            if layer == 1:
                ptp = ectx.enter_context(tc.tile_pool(name="e1_pst", bufs=1, space="PSUM"))
                php = ectx.enter_context(tc.tile_pool(name="e1_ph", bufs=1, space="PSUM"))
            ERW = 128  # elems gathered per er-row read (256B)

            for w in range(NW):
                k = int(kw[w])
                o = int(offs[w])
                base = w * P
                ns = min(P, S - base)

                si = ip.tile([P, k], i32, tag="si")
                nc.sync.dma_start(out=si[:], in_=srcidx_d[:, o : o + k])
                dl = ip.tile([P, k], bf16, tag="dl")
                nc.sync.dma_start(out=dl[:], in_=dstl_d[:, o : o + k])

                dlT = ip.tile([1, k * P], bf16, tag="dlT")
                nc.sync.dma_start(
                    out=dlT[:],
                    in_=bass.AP(tensor=dstlT_d.tensor, offset=o * P,
                                ap=[[1, 1], [1, k * P]]),
                )

                # per-column 128-row gathers (one offset per partition)
                G = gp.tile([P, k * RW], bf16, tag="G")
                for j in range(k):
                    nc.gpsimd.indirect_dma_start(
                        out=G[:, j * RW : (j + 1) * RW],
                        out_offset=None,
                        in_=TBL[:, :],
                        in_offset=IndirectOffsetOnAxis(ap=si[:, j : j + 1], axis=0),
                    )
                if debug_stage >= 2 and layer == 1 and w == 0:
                    nc.sync.dma_start(out=dbg_G_d[:, :], in_=G[:])

                # er for this window's dst slots: sequential rows of LOC
                erwin = ep.tile([P, NH], bf16, tag="erwin")
                nc.vector.memset(erwin[:], 0.0)
                nc.sync.dma_start(out=erwin[:ns, :], in_=LOC[base : base + ns, 0:NH])

                # OHT[slot, edge] = (dstl[edge] == slot): broadcast dstlT down
                # partitions via ones-matmul, compare to per-partition iota
                OHT = op.tile([P, k * P], bf16, tag="OHT")
                for c0 in range(0, k * P, 512):
                    cw = min(512, k * P - c0)
                    dx = pep.tile([P, 512], f32, tag="dx")
                    nc.tensor.matmul(
                        dx[:, :cw], lhsT=ones_row_b[:], rhs=dlT[:, c0 : c0 + cw],
                        start=True, stop=True,
                    )
                    nc.vector.tensor_scalar(
                        out=OHT[:, c0 : c0 + cw], in0=dx[:, :cw],
                        scalar1=iota_c[:, 0:1], scalar2=None, op0=ALU.is_equal,
                    )

                # er per edge via PE: pse = OHT_j^T @ erwin
                pse = pep2.tile([P, k * NH], f32, tag="pse")
                for j in range(k):
                    nc.tensor.matmul(
                        pse[:, j * NH : (j + 1) * NH],
                        lhsT=OHT[:, j * P : (j + 1) * P],
                        rhs=erwin[:],
                        start=True, stop=True,
                    )

                # e = el[src] + er[dst]
                el_view = _ap(G[:], NH, [[RW, k], [1, NH]])
                et = sp.tile([P, k * NH], f32, tag="et")
                nc.vector.tensor_tensor(out=et[:], in0=el_view, in1=pse[:], op=ALU.add)
                ea = sp.tile([P, k * NH], f32, tag="ea")
                nc.scalar.activation(ea[:], et[:], ACTF.Exp)
                eb = sp.tile([P, k * NH], f32, tag="eb")
                nc.scalar.activation(eb[:], et[:], ACTF.Exp, scale=cfg.SLOPE)
                # p -> G's el cols (bf16)
                nc.vector.tensor_tensor(out=el_view, in0=ea[:], in1=eb[:], op=ALU.max)

                # msg = p * h in place
                if layer == 1:
                    h_view = _ap(G[:], 2 * NH, [[RW, k], [DH, HEADS], [1, DH]])
                    p_view = _ap(G[:], NH, [[RW, k], [1, HEADS], [0, DH]])
                else:
                    h_view = _ap(G[:], 2, [[RW, k], [1, NCLS]])
                    p_view = _ap(G[:], 1, [[RW, k], [0, NCLS]])
                nc.vector.tensor_tensor(out=h_view, in0=h_view, in1=p_view, op=ALU.mult)

                # one-hot [edges, slots] per column
                OH = op.tile([P, k * P], bf16, tag="OH")
                nc.vector.tensor_tensor(
                    out=_ap(OH[:], 0, [[P, k], [1, P]]),
                    in0=_ap(iota_b[:], 0, [[0, k], [1, P]]),
                    in1=_ap(dl[:], 0, [[1, k], [0, P]]),
                    op=ALU.is_equal,
                )

                # scatter: psum[slot, 0:NH]=sum p, [NH:MW]=sum p*h
                ps = pp.tile([P, MW], f32, tag="eps")
                for j in range(k):
                    nc.tensor.matmul(
                        ps[:, :],
                        lhsT=OH[:, j * P : (j + 1) * P],
                        rhs=G[:, j * RW + NH : j * RW + NH + MW],
                        start=(j == 0),
                        stop=(j == k - 1),
                    )

                scl = sp.tile([P, NH], f32, tag="scl")
                nc.vector.tensor_scalar(
                    out=scl[:], in0=ps[:, 0:NH], scalar1=1e-30, scalar2=None, op0=ALU.max
                )
                rs = sp.tile([P, NH], f32, tag="rs")
                nc.vector.reciprocal(rs[:], scl[:])

                if layer == 1:
                    h1 = wp.tile([P, HID], f32, tag="h1")
                    nc.vector.tensor_tensor(
                        out=_ap(h1[:], 0, [[DH, HEADS], [1, DH]]),
                        in0=_ap(ps[:], NH, [[DH, HEADS], [1, DH]]),
                        in1=_ap(rs[:], 0, [[1, HEADS], [0, DH]]),
                        op=ALU.mult,
                    )
                    nc.vector.tensor_tensor(out=h1[:], in0=h1[:], in1=bias1[:], op=ALU.add)
                    cur = h1
                    for r in range(2):
                        tmin = wp.tile([P, HID], f32, tag=f"tmin{r}")
                        nc.vector.tensor_scalar(
                            out=tmin[:], in0=cur[:], scalar1=0.0, scalar2=None, op0=ALU.min
                        )
                        nc.scalar.activation(tmin[:], tmin[:], ACTF.Exp)
                        nc.vector.tensor_scalar(
                            out=tmin[:], in0=tmin[:], scalar1=-1.0, scalar2=None, op0=ALU.add
                        )
                        nxt = wp.tile([P, HID], f32, tag=f"helu{r}")
                        nc.vector.tensor_tensor(out=nxt[:], in0=cur[:], in1=tmin[:], op=ALU.max)
                        cur = nxt
                    # h2 = cur.T-transpose (fp32) then @ W2e (bf16)
                    pt = ptp.tile([P, P], f32, tag="pt")
                    nc.tensor.transpose(pt[:], cur[:], identity[:])
                    hT = wp.tile([P, P], bf16, tag="hT")
                    nc.vector.tensor_copy(hT[:], pt[:])
                    ps2 = php.tile([P, F2], f32, tag="ph2")
                    nc.tensor.matmul(
                        ps2[:, :], lhsT=hT[:HID, :], rhs=W2e[:HID, :], start=True, stop=True
                    )
                    t2 = wp.tile([P, R2], bf16, tag="t2")
                    nc.vector.tensor_copy(t2[:ns, 0:1], ps2[:ns, NCLS + 1 : NCLS + 2])
                    nc.vector.tensor_copy(t2[:ns, 1:2], ps2[:ns, NCLS : NCLS + 1])
                    nc.vector.tensor_copy(t2[:ns, 2 : 2 + NCLS], ps2[:ns, 0:NCLS])
                    nc.vector.memset(t2[:ns, 2 + NCLS : R2], 0.0)
                    nc.sync.dma_start(out=t2loc_d[base : base + ns, :], in_=t2[:ns, :])
                else:
                    o2 = wp.tile([P, NCLS], f32, tag="o2")
                    nc.vector.tensor_scalar(
                        out=o2[:], in0=ps[:, 1 : 1 + NCLS], scalar1=rs[:, 0:1], scalar2=None, op0=ALU.mult
                    )
                    nc.vector.tensor_tensor(out=o2[:], in0=o2[:], in1=bias2[:], op=ALU.add)
                    nc.sync.dma_start(out=out_d[base : base + ns, :], in_=o2[:ns, :])

        for _rep in range(repeat):
            with ExitStack() as actx:
                stage_a(actx)

            nc.gpsimd.collective_compute(
                "AllGather", ALU.bypass, replica_groups=[list(range(cfg.C))],
                ins=[t1loc_d[:, :]], outs=[t1full_d[:, :]],
            )
            if debug_stage >= 1:
                nc.sync.dma_start(out=dbg_t1loc_d[:, :], in_=t1loc_d[:, :])
            if debug_stage >= 2:
                nc.sync.dma_start(out=dbg_t1full_d[:, :], in_=t1full_d[:, :])

            with ExitStack() as e1ctx:
                edge_phase(1, e1ctx)
            if debug_stage >= 3:
                nc.sync.dma_start(out=dbg_t2loc_d[:, :], in_=t2loc_d[:, :])

            nc.gpsimd.collective_compute(
                "AllGather", ALU.bypass, replica_groups=[list(range(cfg.C))],
                ins=[t2loc_d[:, :]], outs=[t2full_d[:, :]],
            )

            with ExitStack() as e2ctx:
                edge_phase(2, e2ctx)

    nc.compile()
    return nc


def make_inmaps(cfg: Cfg, inputs: dict, packs):
    x = np.asarray(inputs["x"], np.float32)
    W1 = np.asarray(inputs["W1"], np.float32)
    al1 = np.asarray(inputs["al1"], np.float32)
    ar1 = np.asarray(inputs["ar1"], np.float32)
    b1 = np.asarray(inputs["b1"], np.float32)
    W2 = np.asarray(inputs["W2"], np.float32)
    al2 = np.asarray(inputs["al2"], np.float32)
    ar2 = np.asarray(inputs["ar2"], np.float32)
    b2 = np.asarray(inputs["b2"], np.float32)

    xT = np.ascontiguousarray(x.T)
    W1T = np.ascontiguousarray(W1.T)
    W2T = np.ascontiguousarray(W2.T)
    alar1 = np.zeros((cfg.HID, 2 * cfg.HEADS), np.float32)
    for h in range(cfg.HEADS):
        alar1[h * cfg.DH : (h + 1) * cfg.DH, h] = al1[h]
        alar1[h * cfg.DH : (h + 1) * cfg.DH, cfg.HEADS + h] = ar1[h]
    alar2 = np.stack([al2[0], ar2[0]], axis=1).astype(np.float32)

    in_maps = []
    for c in range(cfg.C):
        srcidx, dstidx, dstl, dstlT = packs[c]
        in_maps.append(
            {
                "xT": xT, "W1": W1, "W1T": W1T, "alar1": alar1, "b1": b1,
                "W2": W2, "W2T": W2T, "alar2": alar2, "b2": b2,
                "srcidx": srcidx, "dstidx": dstidx, "dstl": dstl,
                "dstlT": dstlT,
            }
        )
    return in_maps


def run(cfg: Cfg, inputs: dict, trace: bool = False):
    kw, offs, T, packs = host_prep(cfg, inputs["src"], inputs["dst"])
    nc = build_program(cfg, kw, offs, T)
    in_maps = make_inmaps(cfg, inputs, packs)
    res = run_bass_kernel_spmd(nc, in_maps, core_ids=list(range(cfg.C)), trace=trace)
    out = np.concatenate([res.results[c]["out"] for c in range(cfg.C)], axis=0)
    return out, res


def kernel(**inputs) -> np.ndarray:
    cfg = Cfg()
    out, _ = run(cfg, inputs)
    return out.astype(np.float32)
